# revision 24
# baseline (speedup 1.0000x reference)
"""Convex_f forward on 8 trn2 NeuronCores (pure data parallel over batch).

Math: with y = x + param and the interior 3-point stencils
  Dy[i]    = -y[i-1] + 2 y[i] - y[i+1]          (0 at i = 0, N-1)
  mid_y[i] = 0.5 (y[i-1] + y[i+1])
the reference computes out = y - (Dy > 0) * (y - mid_y) - param.
Since y - mid_y = 0.5 * Dy on the interior, this collapses to
  out[i] = x[i] - relu(ctr - 0.5*up - 0.5*dn)   for 0 < i < N-1
  out[i] = x[i]                                  at i = 0, N-1.

The boundary case is folded into the interior formula by padding each
batch with a halo row at both N-ends host-side: x_halo = +1e30 and
param_halo = 0, so y_halo = +1e30 and the relu argument at the edge
rows is hugely negative -> out = x there.

Per-core layout: partition p holds J=64 consecutive n-rows (x16 K) per
batch, so the stencil shift is a free-dim offset of K elements and every
DMA is one large transfer with 2KiB+ contiguous runs per partition.

Strategy bf16 (default): the rel-err budget is 2e-2, bf16 rounding
costs ~3e-3, so all HBM traffic is bf16 — 25.7 MiB/core instead of
48 MiB/core f32.  DVE tensor_tensor supports the 2x 16-bit perf mode
but scalar_tensor_tensor does not, so the whole stencil is expressed
as plain TT ops with the 0.5 scaling done by ScalarE:
    y = x + p                (DVE TT, 2x)
    h = 0.5*y                (ScalarE activation Copy scale=0.5)
    t = h_up + h_dn          (DVE TT, 2x)
    u = t - p_ctr            (DVE TT, 2x)   [= x_ctr - relu_arg]
    o = min(x_ctr, u)        (DVE TT, 2x)   [= x - relu(relu_arg)]
Loads split over the SP and ACT HWDGE rings; stores on SWDGE.

Strategy pe_y: the previous f32 kernel (kept as a fallback; see git
history of this docstring for details).
"""

import os

import numpy as np

B, N, K = 256, 8192, 16
NCORES = 8
BPC = B // NCORES  # 32 batches per core
P = 128
J = N // P         # 64 n-rows per partition per batch
NP = N + 2         # padded rows per batch
FHB = (J + 2) * K  # 1056 haloed free elems per batch per partition
FIB = J * K        # 1024 interior free elems per batch per partition
BIG = 1.0e30

STRATEGY = os.environ.get("CONVEX_STRATEGY", "tri")

# Overlapped-chunk layout (strategy "tri"): the stencil runs along the
# partition dim via one tridiagonal 128x128 matmul.  Chunks of 128 padded
# rows advance by 126 rows, so partitions 1..126 have both neighbors
# in-chunk; partitions 0/127 produce junk the host discards.
TSTEP = 126
TCH = 66                    # ceil((N + 2) / TSTEP) chunks per batch
TROWS = TCH * TSTEP + 2     # 8318 padded rows per batch host-side
TF = TCH * K                # 1056 free elems per partition per batch
BPI = int(os.environ.get("CONVEX_BPI", "1"))     # batches per iteration
BUFS = int(os.environ.get("CONVEX_BUFS", "10"))
PIPE = int(os.environ.get("CONVEX_PIPE", "2"))   # sw-pipeline the tail op
STORE = os.environ.get("CONVEX_STORE", "sync")   # store DMA issuing engine
LC = int(os.environ.get("CONVEX_LC", "4"))       # load lookahead (iters)
LT = int(os.environ.get("CONVEX_LT", "2"))       # tail lag (iters)

_cache = {}

# Results of the last hardware run (BassKernelResults); test harnesses can
# read exec_time_ns etc. from here after calling kernel().
LAST_RESULTS = None


def _build_nc():
    import concourse.bacc as bacc
    import concourse.bass as bass
    import concourse.mybir as mybir
    from concourse.tile import TileContext

    f32 = mybir.dt.float32
    bf16 = mybir.dt.bfloat16
    AO = mybir.AluOpType
    AF = mybir.ActivationFunctionType
    FH = BPI * FHB
    FI = BPI * FIB

    dt_io = bf16 if STRATEGY in ("bf16", "pe_d", "tri") else f32

    nc = bacc.Bacc()
    if STRATEGY == "tri":
        return _build_tri(nc, bass, mybir)
    x_d = nc.dram_tensor("x", [BPC, NP, K], dt_io, kind="ExternalInput")
    p_d = nc.dram_tensor("p", [BPC, NP, K], dt_io, kind="ExternalInput")
    o_d = nc.dram_tensor("o", [BPC, N, K], dt_io, kind="ExternalOutput")

    def halo_ap(handle, b0):
        # [p, q, f]: partition p reads padded rows [p*J, p*J + J + 2) of
        # batches b0..b0+BPI-1 (overlapping reads across partitions).
        return bass.AP(handle, b0 * NP * K, [[J * K, P], [NP * K, BPI], [1, FHB]])

    def out_ap(handle, b0):
        return bass.AP(handle, b0 * N * K, [[J * K, P], [N * K, BPI], [1, FIB]])

    n_iter = BPC // BPI
    if STRATEGY == "pe_y":
        return _build_pe_y(nc, bass, mybir, x_d, p_d, o_d, halo_ap, out_ap)
    if STRATEGY == "pe_d":
        return _build_pe_d(nc, bass, mybir, x_d, p_d, o_d, halo_ap, out_ap)

    with TileContext(nc) as tc:
        with tc.tile_pool(name="io", bufs=BUFS) as pool:
            pend = []

            def stage_a(it):
                b0 = it * BPI
                x_t = pool.tile([P, FH], bf16, name="x_t")
                p_t = pool.tile([P, FH], bf16, name="p_t")
                y_t = pool.tile([P, FH], bf16, name="y_t")
                h_t = pool.tile([P, FH], bf16, name="h_t")
                u_t = pool.tile([P, FI], bf16, name="u_t")

                nc.sync.dma_start(x_t[:], halo_ap(x_d, b0))
                nc.scalar.dma_start(p_t[:], halo_ap(p_d, b0))

                nc.vector.tensor_tensor(y_t[:], x_t[:], p_t[:], op=AO.add)
                nc.scalar.activation(h_t[:], y_t[:], AF.Copy, scale=0.5)

                h3 = h_t.rearrange("p (q f) -> p q f", q=BPI)
                p3 = p_t.rearrange("p (q f) -> p q f", q=BPI)
                u3 = u_t.rearrange("p (q f) -> p q f", q=BPI)
                # t = h_up + h_dn ; u = t - p_ctr (= x_ctr - relu_arg)
                nc.vector.tensor_tensor(u3[:], h3[:, :, 0:FIB],
                                        h3[:, :, 2 * K:2 * K + FIB], op=AO.add)
                nc.vector.tensor_tensor(u3[:], u3[:], p3[:, :, K:K + FIB],
                                        op=AO.subtract)
                return (it, x_t, u_t)

            def stage_b(state):
                it, x_t, u_t = state
                b0 = it * BPI
                x3 = x_t.rearrange("p (q f) -> p q f", q=BPI)
                u3 = u_t.rearrange("p (q f) -> p q f", q=BPI)
                # o = min(x_ctr, u) = x - relu(relu_arg), in place over u
                nc.vector.tensor_tensor(u3[:], x3[:, :, K:K + FIB], u3[:],
                                        op=AO.min)
                nc.gpsimd.dma_start(out_ap(o_d, b0), u_t[:])

            for it in range(n_iter):
                pend.append(stage_a(it))
                if len(pend) > PIPE:
                    stage_b(pend.pop(0))
            for s in pend:
                stage_b(s)
    nc.finalize()
    return nc


def _build_tri(nc, bass, mybir):
    """Stencil along the partition dim: one tridiagonal PE pass.

    Host shuffles each padded batch (BIG/0 halo rows + BIG tail pad) into
    [P, b, c, k] with padded row index r = c*TSTEP + p, so chunks overlap
    by 2 rows and partitions 1..126 of every chunk have both stencil
    neighbors in-chunk.  Then with T = I - 0.5*(sub+super diagonals):
        y = x + p                 (DVE TT, 2x, bf16)
        d = T @ y                 (PE, ONE pass, f32 PSUM; rows 0/127 junk)
        r = relu(d)               (ScalarE, PSUM -> SBUF bf16)
        o = x - r                 (DVE TT, 2x)
    o is stored in the shuffled layout; the host keeps partitions 1..126
    and inverse-shuffles (1.6% junk traffic).  Per iteration the PE does
    ~1us instead of ~2.3us (6 matmuls + 6 weight loads in pe_d), and all
    DMA runs stay 2112B contiguous per partition.
    """
    import numpy as np
    import ml_dtypes
    from concourse.tile import TileContext

    f32 = mybir.dt.float32
    bf16 = mybir.dt.bfloat16
    AO = mybir.AluOpType
    AF = mybir.ActivationFunctionType

    tri = np.eye(P)
    for i in range(P - 1):
        tri[i + 1, i] = -0.5     # T[q, p]: column p reads rows p-1, p, p+1
        tri[i, i + 1] = -0.5
    tri_d = nc.inline_tensor(tri.astype(ml_dtypes.bfloat16), name="tri")

    x_d = nc.dram_tensor("x", [P, BPC, TF], bf16, kind="ExternalInput")
    p_d = nc.dram_tensor("p", [P, BPC, TF], bf16, kind="ExternalInput")
    o_d = nc.dram_tensor("o", [P, BPC, TF], bf16, kind="ExternalOutput")

    def slab_ap(handle, b):
        return bass.AP(handle, b * TF, [[BPC * TF, P], [1, TF]])

    with TileContext(nc) as tc:
        with (
            tc.tile_pool(name="const", bufs=1) as cpool,
            tc.tile_pool(name="io", bufs=BUFS) as pool,
            tc.tile_pool(name="ps", bufs=2, space="PSUM") as pspool,
        ):
            tri_t = cpool.tile([P, P], bf16, name="tri_t")
            nc.sync.dma_start(tri_t[:], tri_d.ap())

            # Explicit 3-stage software pipeline.  Every sequencer's
            # in-order stream must reach its DMA doorbells with the
            # dependencies already satisfied, otherwise the transfer
            # latency lands on the serial per-iteration loop (Scalar
            # issuing p_{i+1} right after relu_i put the whole
            # load->add->matmul->relu chain on the DMA issue path).
            # Loads run LC iterations ahead of compute; the subtract +
            # store trail LT iterations behind.
            def stage_load(b):
                x_t = pool.tile([P, TF], bf16, name="x_t")
                p_t = pool.tile([P, TF], bf16, name="p_t")
                nc.sync.dma_start(x_t[:], slab_ap(x_d, b))
                nc.scalar.dma_start(p_t[:], slab_ap(p_d, b))
                return (b, x_t, p_t)

            def stage_comp(state):
                b, x_t, p_t = state
                y_t = pool.tile([P, TF], bf16, name="y_t")
                r_t = pool.tile([P, TF], bf16, name="r_t")
                nc.vector.tensor_tensor(y_t[:], x_t[:], p_t[:], op=AO.add)
                ps = pspool.tile([P, TF], f32, name="ps")
                for c0 in range(0, TF, 512):
                    c1 = min(c0 + 512, TF)
                    nc.tensor.matmul(ps[:, c0:c1], tri_t[:], y_t[:, c0:c1],
                                     start=True, stop=True)
                nc.scalar.activation(r_t[:], ps[:], AF.Relu)
                return (b, x_t, r_t)

            def store_ap(b):
                # only the 126 valid partitions (rows 0/127 are junk)
                return bass.AP(o_d, BPC * TF + b * TF,
                               [[BPC * TF, P - 2], [1, TF]])

            def stage_tail(state):
                b, x_t, r_t = state
                # o = x - relu(d), in place over r
                nc.vector.tensor_tensor(r_t[:], x_t[:], r_t[:], op=AO.subtract)
                if STORE == "gpsimd":
                    nc.gpsimd.dma_start(store_ap(b), r_t[1:P - 1])
                elif STORE == "scalar":
                    nc.scalar.dma_start(store_ap(b), r_t[1:P - 1])
                else:
                    nc.sync.dma_start(store_ap(b), r_t[1:P - 1])

            loaded, computed = [], []
            for i in range(BPC + LC + LT):
                if i < BPC:
                    loaded.append(stage_load(i))
                if i >= LC and i - LC < BPC:
                    computed.append(stage_comp(loaded.pop(0)))
                if i >= LC + LT:
                    stage_tail(computed.pop(0))
    nc.finalize()
    return nc


def _build_pe_d(nc, bass, mybir, x_d, p_d, o_d, halo_ap, out_ap):
    """bf16 I/O with the stencil on the TensorEngine.

    DVE tensor_tensor is capped at the 2x_1p perf mode (~(58+FD/2)/0.96GHz
    per op), so four DVE passes/iter (~77us/core) dominate the DMA floor
    (~72us).  The PE is errata-free at 2.4GHz and a free-dim shift is just
    a column offset on the moving operand, so compute
        d = I*y_ctr + (-0.5I)*y_up + (-0.5I)*y_dn   (PE, f32 PSUM)
        r = relu(d)                                 (ScalarE, PSUM->SBUF bf16)
        o = x_ctr - r                               (DVE TT, 2x)
    leaving DVE only y = x + p and the final subtract (~39us).  Weights
    are ordered ctr,ctr,up,dn,up,dn per iter so only 2 LdWeights happen.
    """
    import numpy as np
    import ml_dtypes
    from concourse.tile import TileContext

    f32 = mybir.dt.float32
    bf16 = mybir.dt.bfloat16
    AO = mybir.AluOpType
    AF = mybir.ActivationFunctionType
    FH = BPI * FHB
    FI = BPI * FIB
    n_iter = BPC // BPI
    CH = 512  # psum accumulation chunk (one bank)

    ident_d = nc.inline_tensor(
        np.eye(P, dtype=ml_dtypes.bfloat16), name="ident")
    nhalf_d = nc.inline_tensor(
        (np.eye(P) * -0.5).astype(ml_dtypes.bfloat16), name="nhalf")

    with TileContext(nc) as tc:
        with (
            tc.tile_pool(name="const", bufs=1) as cpool,
            tc.tile_pool(name="io", bufs=BUFS) as pool,
            tc.tile_pool(name="ps", bufs=4, space="PSUM") as pspool,
        ):
            ident_t = cpool.tile([P, P], bf16, name="ident_t")
            nhalf_t = cpool.tile([P, P], bf16, name="nhalf_t")
            nc.sync.dma_start(ident_t[:], ident_d.ap())
            nc.sync.dma_start(nhalf_t[:], nhalf_d.ap())

            pend = []

            def stage_a(it):
                b0 = it * BPI
                x_t = pool.tile([P, FH], bf16, name="x_t")
                p_t = pool.tile([P, FH], bf16, name="p_t")
                y_t = pool.tile([P, FH], bf16, name="y_t")
                r_t = pool.tile([P, FI], bf16, name="r_t")

                # descriptor generation costs the issuing sequencer ~680ns
                # per dma_start, so spread the three DMAs over two issuing
                # engines: x + store on Sync, p on Scalar (ACT ring).
                nc.sync.dma_start(x_t[:], halo_ap(x_d, b0))
                nc.scalar.dma_start(p_t[:], halo_ap(p_d, b0))
                nc.vector.tensor_tensor(y_t[:], x_t[:], p_t[:], op=AO.add)

                for q in range(BPI):
                    qo = q * FHB
                    ps = pspool.tile([P, FIB], f32, name="ps")
                    # d = y_ctr - 0.5*y_up - 0.5*y_dn, PE-accumulated in
                    # f32 PSUM; ctr chunks first so the identity weights
                    # load once, then the -0.5 weights once.
                    for c0 in range(0, FIB, CH):
                        nc.tensor.matmul(ps[:, c0:c0 + CH], ident_t[:],
                                         y_t[:, qo + K + c0:qo + K + c0 + CH],
                                         start=True, stop=False)
                    for c0 in range(0, FIB, CH):
                        nc.tensor.matmul(ps[:, c0:c0 + CH], nhalf_t[:],
                                         y_t[:, qo + c0:qo + c0 + CH],
                                         start=False, stop=False)
                        nc.tensor.matmul(ps[:, c0:c0 + CH], nhalf_t[:],
                                         y_t[:, qo + 2 * K + c0:qo + 2 * K + c0 + CH],
                                         start=False, stop=True)
                    # r = relu(d): PSUM f32 -> SBUF bf16 on ScalarE
                    nc.scalar.activation(
                        r_t[:, q * FIB:(q + 1) * FIB], ps[:], AF.Relu)
                return (it, x_t, r_t)

            def stage_b(state):
                it, x_t, r_t = state
                b0 = it * BPI
                x3 = x_t.rearrange("p (q f) -> p q f", q=BPI)
                r3 = r_t.rearrange("p (q f) -> p q f", q=BPI)
                # o = x_ctr - relu(d), in place over r
                nc.vector.tensor_tensor(r3[:], x3[:, :, K:K + FIB], r3[:],
                                        op=AO.subtract)
                # store on HWDGE too: SWDGE store rows ran at ~13 GB/s/queue
                # vs ~26 for HWDGE.  Issued PIPE iterations late so the wait
                # on sub_i can't head-of-line-block the next loads.
                if STORE == "gpsimd":
                    nc.gpsimd.dma_start(out_ap(o_d, b0), r_t[:])
                elif STORE == "scalar":
                    nc.scalar.dma_start(out_ap(o_d, b0), r_t[:])
                else:
                    nc.sync.dma_start(out_ap(o_d, b0), r_t[:])

            for it in range(n_iter):
                pend.append(stage_a(it))
                if len(pend) > PIPE:
                    stage_b(pend.pop(0))
            for s in pend:
                stage_b(s)
    nc.finalize()
    return nc


def _build_pe_y(nc, bass, mybir, x_d, p_d, o_d, halo_ap, out_ap):
    """y = x + param on the TensorEngine (identity-matmul accumulate into
    PSUM), then per batch on DVE (each op reads at most one PSUM operand):
        u1 = 0.5*y_up - p_ctr
        u  = 0.5*y_dn + u1         (= x_ctr - d, with d the relu argument)
        o  = min(x_ctr, u)         (= x - relu(d))
    No relu, no PSUM->SBUF copy, no y-add on DVE. Loads split over the SP
    and ACT HWDGE rings; stores on SWDGE (GpSimd is otherwise idle).
    """
    import numpy as np
    from concourse.tile import TileContext

    f32 = mybir.dt.float32
    AO = mybir.AluOpType
    FH = BPI * FHB
    FI = BPI * FIB
    n_iter = BPC // BPI

    ident_d = nc.inline_tensor(np.eye(P, dtype=np.float32), name="ident")

    with TileContext(nc) as tc:
        with (
            tc.tile_pool(name="const", bufs=1) as cpool,
            tc.tile_pool(name="io", bufs=BUFS) as pool,
            tc.tile_pool(name="ps", bufs=2, space="PSUM") as pspool,
        ):
            ident_t = cpool.tile([P, P], f32, name="ident_t")
            nc.sync.dma_start(ident_t[:], ident_d.ap())

            pend = []

            def stage_a(it):
                b0 = it * BPI
                x_t = pool.tile([P, FH], f32, name="x_t")
                p_t = pool.tile([P, FH], f32, name="p_t")
                u_t = pool.tile([P, FI], f32, name="u_t")

                nc.sync.dma_start(x_t[:], halo_ap(x_d, b0))
                nc.scalar.dma_start(p_t[:], halo_ap(p_d, b0))

                p3 = p_t.rearrange("p (q f) -> p q f", q=BPI)
                u3 = u_t.rearrange("p (q f) -> p q f", q=BPI)

                for q in range(BPI):
                    ps = pspool.tile([P, FHB], f32, name="ps")
                    qo = q * FHB
                    # y = x + p, accumulated on the PE per <=512-col chunk
                    for c0 in range(0, FHB, 512):
                        c1 = min(c0 + 512, FHB)
                        nc.tensor.matmul(ps[:, c0:c1], ident_t[:],
                                         x_t[:, qo + c0:qo + c1],
                                         start=True, stop=False)
                        nc.tensor.matmul(ps[:, c0:c1], ident_t[:],
                                         p_t[:, qo + c0:qo + c1],
                                         start=False, stop=True)
                    uq = u3[:, q, :]
                    # u1 = 0.5*y_up - p_ctr ; u = 0.5*y_dn + u1
                    nc.vector.scalar_tensor_tensor(
                        uq, ps[:, 0:FIB], 0.5, p3[:, q, K:K + FIB],
                        AO.mult, AO.subtract)
                    nc.vector.scalar_tensor_tensor(
                        uq, ps[:, 2 * K:2 * K + FIB], 0.5, uq,
                        AO.mult, AO.add)
                return (it, x_t, u_t)

            def stage_b(state):
                it, x_t, u_t = state
                b0 = it * BPI
                o_t = pool.tile([P, FI], f32, name="o_t")
                x3 = x_t.rearrange("p (q f) -> p q f", q=BPI)
                o3 = o_t.rearrange("p (q f) -> p q f", q=BPI)
                u3 = u_t.rearrange("p (q f) -> p q f", q=BPI)
                # o = min(x_ctr, u) = x - relu(d)
                nc.vector.tensor_tensor(o3[:], x3[:, :, K:K + FIB], u3[:],
                                        op=AO.min)
                nc.gpsimd.dma_start(out_ap(o_d, b0), o_t[:])

            for it in range(n_iter):
                pend.append(stage_a(it))
                if len(pend) > PIPE:
                    stage_b(pend.pop(0))
            for s in pend:
                stage_b(s)
    nc.finalize()
    return nc


def _shuffle_tri(x, param):
    # -> per-core slabs [NCORES, P, BPC, TF]: X[n, p, b, c*K+k] holds
    # padded row c*TSTEP + p of batch b (rows 0 and N+1.. are BIG halo/pad)
    import ml_dtypes
    from numpy.lib.stride_tricks import as_strided

    dt = ml_dtypes.bfloat16
    x = np.ascontiguousarray(x, dtype=np.float32).reshape(NCORES, BPC, N, K)
    param = np.ascontiguousarray(param, dtype=np.float32).reshape(NCORES, BPC, N, K)
    out = []
    for arr, halo in ((x, BIG), (param, 0.0)):
        ap = np.empty((NCORES, BPC, TROWS, K), dtype=dt)
        ap[:, :, 1:N + 1] = arr.astype(dt)
        ap[:, :, 0] = dt(halo)
        ap[:, :, N + 1:] = dt(BIG) if halo else dt(0.0)
        s = ap.strides
        v = as_strided(
            ap, shape=(NCORES, BPC, TCH, P, K),
            strides=(s[0], s[1], TSTEP * s[2], s[2], s[3]))
        out.append(np.ascontiguousarray(
            v.transpose(0, 3, 1, 2, 4)).reshape(NCORES, P, BPC, TF))
    return out[0], out[1]


def _unshuffle_tri(cores_out):
    # cores_out: list of [P, BPC, TF] bf16 -> [B, N, K] f32
    full = np.empty((B, N, K), dtype=np.float32)
    for ci, o in enumerate(cores_out):
        o4 = np.asarray(o).reshape(P, BPC, TCH, K)[1:P - 1]
        o4 = o4.transpose(1, 2, 0, 3).reshape(BPC, TCH * (P - 2), K)
        full[ci * BPC:(ci + 1) * BPC] = o4[:, :N].astype(np.float32)
    return full


def _pad_inputs(x, param):
    # -> per-core padded slabs, shape [NCORES, BPC, NP, K]
    if STRATEGY in ("bf16", "pe_d"):
        import ml_dtypes
        dt = ml_dtypes.bfloat16
    else:
        dt = np.float32
    x = np.ascontiguousarray(x, dtype=np.float32).reshape(NCORES, BPC, N, K)
    param = np.ascontiguousarray(param, dtype=np.float32).reshape(NCORES, BPC, N, K)
    xp = np.empty((NCORES, BPC, NP, K), dtype=dt)
    pp = np.empty((NCORES, BPC, NP, K), dtype=dt)
    xp[:, :, 1:N + 1] = x.astype(dt) if dt is not np.float32 else x
    xp[:, :, 0] = dt(BIG)
    xp[:, :, N + 1] = dt(BIG)
    pp[:, :, 1:N + 1] = param.astype(dt) if dt is not np.float32 else param
    pp[:, :, 0] = dt(0.0)
    pp[:, :, N + 1] = dt(0.0)
    return xp, pp


def kernel(x: np.ndarray, param: np.ndarray) -> np.ndarray:
    global LAST_RESULTS
    from concourse.bass_utils import run_bass_kernel_spmd

    if "nc" not in _cache:
        _cache["nc"] = _build_nc()
    nc = _cache["nc"]

    if STRATEGY == "tri":
        xp, pp = _shuffle_tri(x, param)
    else:
        xp, pp = _pad_inputs(x, param)
    in_maps = [{"x": xp[c], "p": pp[c]} for c in range(NCORES)]

    trace = bool(os.environ.get("BASS_TRACE"))
    res = run_bass_kernel_spmd(
        nc, in_maps, core_ids=list(range(NCORES)), trace=trace
    )
    LAST_RESULTS = res
    if STRATEGY == "tri":
        return _unshuffle_tri([res.results[c]["o"] for c in range(NCORES)])
    out = np.concatenate([res.results[c]["o"] for c in range(NCORES)], axis=0)
    return out.reshape(B, N, K).astype(np.float32)
